# revision 15
# baseline (speedup 1.0000x reference)
"""Transformer block (pre-norm attn + MLP) on 8 NeuronCores, data-parallel over batch.

Full inputs in, full outputs out. Each core processes one batch element
x[i] : [1024, 768] through an identical Bass/Tile kernel.

v2: single continuous PE stream. Differences vs v1:
  - warm-up matmul burst so HAM un-throttles before real work
  - LN1 pipelined per token tile with v-generation (PE dense from ~2us)
  - qk-gen interleaved into the attention pair loop (fills exp-wait bubbles)
  - attention pairs ordered nh-major so proj+LN2 of the first token half
    overlap the second half's ACT-bound stretch
  - softmax normalization via reciprocal_approx_fast (5x faster than
    reciprocal, which was 3.3us per pair and paced the whole phase)
  - LN rstd via Ln+Exp on ACT (stays in the natural_log_exp table set; no
    table thrash against the attention exp) and the normalize itself on DVE
  - MLP with 512-wide token groups (half the instruction count, bigger
    gelu batches); fc2 accumulates into a single [P, 6, 512] psum tile
  - x and x1 round-trip through DRAM instead of living in SBUF

Host-side exact refactoring (unchanged from v1):
  - LN gains fold into the following matmul weights: diag(g) @ W.
  - LN biases fold into: per-column bias on q/k, b_proj_eff, b_fc1_eff.
  - w_proj rows re-laid-out head-aligned (row 0 of each 128-block pairs with
    the attention colsum row; zero).
  - Weights cast to bf16 on host; matmuls bf16 x bf16 with fp32 PSUM.
"""
import numpy as np
import ml_dtypes

import concourse.bass as bass
from concourse import bacc, mybir
from concourse.bass_utils import run_bass_kernel_spmd
from concourse.masks import make_identity
from concourse.tile import TileContext

P = 128
N = 1024          # tokens per core (batch element)
C = 768           # model dim
H = 8             # heads
DH = C // H       # 96
DFF = 4 * C       # 3072
NT = N // P       # 8 token tiles
KT = C // P       # 6 feature tiles
FFT = DFF // P    # 24 ff tiles
NH = 2            # halves of the token axis for attention
NC_ = N // NH     # 512
EPS = 1e-5
SCALE = DH ** -0.5
VW = DH           # per-head v width (plus a leading ones column)
QW = 512          # MLP token-group width

F32 = mybir.dt.float32
BF16 = mybir.dt.bfloat16
AF = mybir.ActivationFunctionType
OP = mybir.AluOpType

_CACHED = {}


def build(taps=()):
    nc = bacc.Bacc("TRN2", debug=False)

    x_d = nc.dram_tensor("x", [N, C], F32, kind="ExternalInput")
    wqkv_d = nc.dram_tensor("w_qkv_e", [C, 3 * C], BF16, kind="ExternalInput")
    wproj_d = nc.dram_tensor("w_proj_p", [H * P, C], BF16, kind="ExternalInput")
    wfc1_d = nc.dram_tensor("w_fc1_e", [C, DFF], BF16, kind="ExternalInput")
    wfc2_d = nc.dram_tensor("w_fc2", [DFF, C], BF16, kind="ExternalInput")
    qkb_d = nc.dram_tensor("qk_bias", [P, 2 * H], F32, kind="ExternalInput")
    bp_d = nc.dram_tensor("b_proj_e", [C], F32, kind="ExternalInput")
    bf1_d = nc.dram_tensor("b_fc1_e", [DFF], F32, kind="ExternalInput")
    bf2_d = nc.dram_tensor("b_fc2", [C], F32, kind="ExternalInput")
    y_d = nc.dram_tensor("y", [N, C], F32, kind="ExternalOutput")
    dbg_cs = nc.dram_tensor("dbg_cs", [2 * H, NC_], F32, kind="ExternalOutput")
    dbg_csr = nc.dram_tensor("dbg_csr", [2 * H, NC_], F32, kind="ExternalOutput")

    tap_d = {}
    for name, shape, dt in [
        ("h_fm", [C, N], BF16),
        ("q_fm", [H * P, N], BF16),
        ("k_fm", [H * P, N], BF16),
        ("v_ext", [N, H * (VW + 1)], BF16),
        ("o_fm", [H * P, N], BF16),
        ("x1", [N, C], F32),
        ("h2_fm", [C, N], BF16),
    ]:
        if name in taps:
            tap_d[name] = nc.dram_tensor(
                "tap_" + name, shape, dt, kind="ExternalOutput"
            )

    def bcast_row(dram_t, width):
        return bass.AP(tensor=dram_t, offset=0, ap=[[0, P], [1, width]])

    with TileContext(nc) as tc:
        # ---------------- pools: LEFT stack ----------------
        consts = tc.alloc_tile_pool(name="consts", bufs=1, side="left")
        lnp = tc.alloc_tile_pool(name="lnp", bufs=2, side="left")
        fstage = tc.alloc_tile_pool(name="fstage", bufs=3, side="left")
        h2p = tc.alloc_tile_pool(name="h2p", bufs=1, side="left")
        wqkvp = tc.alloc_tile_pool(name="wqkvp", bufs=1, side="left")
        hfmp = tc.alloc_tile_pool(name="hfmp", bufs=1, side="left")
        wqkvvp = tc.alloc_tile_pool(name="wqkvvp", bufs=1, side="left")
        # ---------------- pools: RIGHT stack ----------------
        x1p = tc.alloc_tile_pool(name="x1p", bufs=1, side="right")
        qkvpool = tc.alloc_tile_pool(name="qkvpool", bufs=1, side="right")
        opool = tc.alloc_tile_pool(name="opool", bufs=1, side="right")
        epool = tc.alloc_tile_pool(name="epool", bufs=2, side="right")
        rrow = tc.alloc_tile_pool(name="rrow", bufs=2, side="right")
        # ---------------- pools: PSUM ----------------
        ps = tc.alloc_tile_pool(name="ps", bufs=3, space="PSUM")
        sps = tc.alloc_tile_pool(name="sps", bufs=2, space="PSUM")

        # ---------- constants ----------
        ident = consts.tile([P, P], BF16)
        make_identity(nc, ident)
        eps_t = consts.tile([P, 1], F32)
        nc.vector.memset(eps_t, EPS)
        qkb = consts.tile([P, 2 * H], F32)
        nc.gpsimd.dma_start(qkb[:], qkb_d[:, :])
        bf1c = consts.tile([P, FFT], F32)
        nc.gpsimd.dma_start(bf1c[:], bf1_d.rearrange("(t p) -> p t", p=P))
        bpb = consts.tile([P, C], F32)
        nc.gpsimd.dma_start(bpb[:], bcast_row(bp_d, C))
        bf2b = consts.tile([P, C], F32)
        nc.gpsimd.dma_start(bf2b[:], bcast_row(bf2_d, C))

        # ---------- big tiles ----------
        h_fm = hfmp.tile([P, KT, N], BF16)
        wqkv = wqkvp.tile([P, KT, 2 * C], BF16)
        wqkv_v = wqkvvp.tile([P, KT, C], BF16)
        h2_fm = h2p.tile([P, KT, N], BF16)
        x1_tok = x1p.tile([P, NT, C], F32)
        q_fm = qkvpool.tile([P, H, N], BF16)
        k_fm = qkvpool.tile([P, H, N], BF16)
        v_ext = qkvpool.tile([P, NT, H, VW + 1], BF16)
        o_fm = opool.tile([P, H, N], BF16)

        # ---------- weight DMAs (sync queue, in need-order) ----------
        # v columns of wqkv first (phase A consumes them), then q, then k.
        wqkv_r = wqkv_d.rearrange("(kt p) o -> p kt o", p=P)
        for c0 in range(0, C, 512):
            cw = min(512, C - c0)
            nc.sync.dma_start(
                wqkv_v[:, :, c0:c0 + cw], wqkv_r[:, :, 2 * C + c0:2 * C + c0 + cw]
            )
        for c0 in range(0, 2 * C, 512):
            nc.sync.dma_start(
                wqkv[:, :, c0:c0 + 512], wqkv_r[:, :, c0:c0 + 512]
            )

        # ---------- pad memsets (DVE/gpsimd, once) ----------
        nc.vector.memset(q_fm[DH:P, :, :], 0.0)
        nc.vector.memset(k_fm[DH:P, :, :], 0.0)
        nc.vector.memset(o_fm[DH:P, :, :], 0.0)
        nc.gpsimd.memset(v_ext[:, :, :, 0], 1.0)

        # ---------- HAM warm-up burst (~3.5us of tiny matmuls) ----------
        warm = ps.tile([P, P], F32, tag="ps")
        for i in range(36):
            nc.tensor.matmul(warm[:, :P], ident[:], ident[:],
                             start=True, stop=True)

        # ---------- helpers ----------
        def layernorm_tile(x_ap, h_tile):
            st = lnp.tile([P, 3, nc.vector.BN_STATS_DIM], F32, tag="bnst")
            for i in range(3):
                nc.vector.bn_stats(
                    out=st[:, i, :], in_=x_ap[:, i * 256:(i + 1) * 256]
                )
            mv = lnp.tile([P, nc.vector.BN_AGGR_DIM], F32, tag="bnmv")
            nc.vector.bn_aggr(out=mv[:], in_=st[:])
            # rstd = exp(-0.5 * ln(var + eps)); stays in the ln/exp table set
            lnv = lnp.tile([P, 1], F32, tag="lnv")
            nc.scalar.activation(
                out=lnv[:], in_=mv[:, 1:2], func=AF.Ln,
                bias=eps_t[:], scale=1.0,
            )
            rstd = lnp.tile([P, 1], F32, tag="rstd")
            nc.scalar.activation(
                out=rstd[:], in_=lnv[:], func=AF.Exp,
                bias=0.0, scale=-0.5,
            )
            # h = (x - mean) * rstd on DVE
            nc.vector.tensor_scalar(
                out=h_tile[:], in0=x_ap[:],
                scalar1=mv[:, 0:1], scalar2=rstd[:],
                op0=OP.subtract, op1=OP.mult,
            )

        def transpose_into(h_tile, dst_fm, nt):
            for kt in range(KT):
                tp = ps.tile([P, P], BF16, tag="ps", name=f"tp_{nt}_{kt}")
                nc.tensor.transpose(
                    tp[:], h_tile[:, kt * P:(kt + 1) * P], ident[:]
                )
                nc.vector.tensor_copy(
                    dst_fm[:, kt, nt * P:(nt + 1) * P], tp[:]
                )

        # ---------- phase A: LN1 + transpose + v-gen, per token tile ----------
        for nt in range(NT):
            x_t = fstage.tile([P, C], F32, tag="fs", name=f"xa_{nt}")
            nc.sync.dma_start(x_t[:], x_d[nt * P:(nt + 1) * P, :])
            h_t = lnp.tile([P, C], BF16, tag="h1")
            layernorm_tile(x_t[:], h_t)
            transpose_into(h_t, h_fm, nt)
            for half in range(2):  # 4 heads (384 cols) per psum
                pv = ps.tile([P, NC_], F32, tag="ps", name=f"pv_{nt}_{half}")
                c0 = half * 4 * DH
                for kt in range(KT):
                    nc.tensor.matmul(
                        pv[:, :4 * DH],
                        h_fm[:, kt, nt * P:(nt + 1) * P],
                        wqkv_v[:, kt, c0:c0 + 4 * DH],
                        start=(kt == 0), stop=(kt == KT - 1),
                    )
                nc.vector.tensor_copy(
                    v_ext[:, nt, half * 4:(half + 1) * 4, 1:VW + 1],
                    pv[:, :4 * DH].rearrange("p (h d) -> p h d", d=DH),
                )

        if "h_fm" in tap_d:
            nc.sync.dma_start(
                tap_d["h_fm"].rearrange("(kt p) n -> p kt n", p=P), h_fm[:]
            )
        if "v_ext" in tap_d:
            nc.sync.dma_start(
                tap_d["v_ext"].rearrange(
                    "(nt p) (h w) -> p nt h w", p=P, w=VW + 1),
                v_ext[:],
            )

        wqkvvp.release()

        # ---------- phase B/C: qk-gen interleaved with attention ----------
        def emit_qk_group(h, which, nh):
            """One psum group: q or k for head h, token half nh."""
            col0 = which * C + h * DH
            dst = q_fm if which == 0 else k_fm
            pq = ps.tile([P, NC_], F32, tag="ps", name=f"qk_{h}_{which}_{nh}")
            for kt in range(KT):
                nc.tensor.matmul(
                    pq[:DH, :],
                    wqkv[:, kt, col0:col0 + DH],
                    h_fm[:, kt, nh * NC_:(nh + 1) * NC_],
                    start=(kt == 0), stop=(kt == KT - 1),
                )
            nc.vector.tensor_scalar_add(
                dst[:DH, h, nh * NC_:(nh + 1) * NC_],
                pq[:DH, :],
                qkb[:DH, which * H + h:which * H + h + 1],
            )

        def emit_scores_part(h, nh, mt2, e_t):
            ps_s = sps.tile([P, 2, NC_], F32, tag="S",
                            name=f"s_{h}_{nh}_{mt2}")
            for sub in range(2):
                nc.tensor.matmul(
                    ps_s[:, sub, :],
                    k_fm[:, h, (2 * mt2 + sub) * P:(2 * mt2 + sub + 1) * P],
                    q_fm[:, h, nh * NC_:(nh + 1) * NC_],
                    start=True, stop=True,
                )
            nc.scalar.activation(
                out=e_t[:, 2 * mt2:2 * mt2 + 2, :], in_=ps_s[:],
                func=AF.Exp, bias=0.0, scale=SCALE,
            )

        def emit_pv(h, nh, e_t):
            po = ps.tile([P, NC_], F32, tag="ps", name=f"po_{h}_{nh}")
            for mt in range(NT):
                nc.tensor.matmul(
                    po[:VW + 1, :],
                    v_ext[:, mt, h, :],
                    e_t[:, mt, :],
                    start=(mt == 0), stop=(mt == NT - 1),
                )
            # softmax normalize: rinv = 1/colsum, broadcast, multiply
            cs = rrow.tile([1, NC_], F32, tag="cs", name=f"cs_{h}_{nh}")
            nc.vector.tensor_copy(cs[0:1, :], po[0:1, :])
            nc.sync.dma_start(dbg_csr[nh * H + h:nh * H + h + 1, :], cs[0:1, :])
            nc.vector.reciprocal_approx_fast(out=cs[0:1, :], in_=cs[0:1, :])
            nc.sync.dma_start(dbg_cs[nh * H + h:nh * H + h + 1, :], cs[0:1, :])
            rb = rrow.tile([P, NC_], F32, tag="rb", name=f"rb_{h}_{nh}")
            nc.gpsimd.partition_broadcast(rb[:VW + 1, :], cs[0:1, :])
            nc.vector.tensor_mul(
                o_fm[0:VW + 1, h, nh * NC_:(nh + 1) * NC_],
                po[0:VW + 1, :], rb[0:VW + 1, :],
            )

        def emit_proj(nt):
            x_t2 = fstage.tile([P, C], F32, tag="fs", name=f"xb_{nt}")
            nc.sync.dma_start(x_t2[:], x_d[nt * P:(nt + 1) * P, :])
            x1_t = x1_tok[:, nt, :]
            for c0, cw in ((0, 512), (512, 256)):
                pj = ps.tile([P, NC_], F32, tag="ps", name=f"pj_{nt}_{c0}")
                for hb in range(H):
                    nc.tensor.matmul(
                        pj[:, :cw],
                        o_fm[:, hb, nt * P:(nt + 1) * P],
                        wproj[:, hb, c0:c0 + cw],
                        start=(hb == 0), stop=(hb == H - 1),
                    )
                nc.vector.tensor_add(
                    x1_t[:, c0:c0 + cw], pj[:, :cw], x_t2[:, c0:c0 + cw]
                )
            nc.vector.tensor_add(x1_t[:], x1_t[:], bpb[:])
            if "x1" in tap_d:
                nc.sync.dma_start(
                    tap_d["x1"][nt * P:(nt + 1) * P, :], x1_t[:]
                )
            return x1_t

        def emit_ln2(nt, x1_t):
            h_t = lnp.tile([P, C], BF16, tag="h1", name=f"h2_{nt}")
            layernorm_tile(x1_t[:], h_t)
            transpose_into(h_t, h2_fm, nt)

        pairs = [(h, nh) for nh in range(NH) for h in range(H)]
        # scores(h, nh) needs q(h, nh) and k(h, BOTH halves): emit per head
        # [q-nh0, k-nh0, k-nh1, q-nh1], 4 groups per pair iteration.
        fill_q = [g for h in range(2, H)
                  for g in [(h, 0, 0), (h, 1, 0), (h, 1, 1), (h, 0, 1)]]
        qk_done = set()

        def emit_qk_group_tracked(h, wh, nh):
            emit_qk_group(h, wh, nh)
            qk_done.add((h, wh, nh))

        for h in range(2):
            for nh in range(NH):
                for wh in range(2):
                    emit_qk_group_tracked(h, wh, nh)

        prev = None
        wprojp = None
        wproj = None
        for i, (h, nh) in enumerate(pairs):
            # scores for (h, nh) need q(h, nh) and k(h, 0) AND k(h, 1)
            assert {(h, 0, nh), (h, 1, 0), (h, 1, 1)} <= qk_done, (h, nh)
            e_t = epool.tile([P, NT, NC_], BF16, tag="E", name=f"e_{h}_{nh}")
            for mt2 in range(NT // 2):
                emit_scores_part(h, nh, mt2, e_t)
                if fill_q:
                    hq, wq, nq = fill_q.pop(0)
                    emit_qk_group_tracked(hq, wq, nq)
            if not fill_q and wprojp is None:
                # qk-gen fully emitted: free h_fm + wqkv; bring in w_proj, wfc1
                hfmp.release()
                wqkvp.release()
                wprojp = tc.alloc_tile_pool(name="wprojp", bufs=1, side="right")
                wproj = wprojp.tile([P, H, C], BF16)
                nc.sync.dma_start(
                    wproj[:], wproj_d.rearrange("(hb p) c -> p hb c", p=P)
                )
                wfc1p = tc.alloc_tile_pool(name="wfc1p", bufs=1, side="left")
                wfc1 = wfc1p.tile([P, KT, DFF], BF16)
                nc.sync.dma_start(
                    wfc1[:], wfc1_d.rearrange("(kt p) f -> p kt f", p=P)
                )
            if prev is not None:
                emit_pv(*prev)
            prev = (h, nh, e_t)
            # after all nh=0 pairs are normalized, overlap proj+LN2 of the
            # first token half with the second half's attention
            if 9 <= i <= 12:
                nt = i - 9
                emit_ln2(nt, emit_proj(nt))
        emit_pv(*prev)
        for nt in range(4, NT):
            emit_ln2(nt, emit_proj(nt))

        if "q_fm" in tap_d:
            nc.sync.dma_start(
                tap_d["q_fm"].rearrange("(h p) n -> p h n", p=P), q_fm[:]
            )
        if "k_fm" in tap_d:
            nc.sync.dma_start(
                tap_d["k_fm"].rearrange("(h p) n -> p h n", p=P), k_fm[:]
            )
        if "o_fm" in tap_d:
            nc.sync.dma_start(
                tap_d["o_fm"].rearrange("(h p) n -> p h n", p=P), o_fm[:]
            )
        if "h2_fm" in tap_d:
            nc.sync.dma_start(
                tap_d["h2_fm"].rearrange("(kt p) n -> p kt n", p=P), h2_fm[:]
            )

        # ---------- release attention-era pools ----------
        sps.release()
        ps.release()
        wprojp.release()
        rrow.release()
        epool.release()
        opool.release()
        qkvpool.release()

        # ---------- phase D: MLP ----------
        wfc2s = tc.alloc_tile_pool(name="wfc2s", bufs=8, side="left")
        gpool = tc.alloc_tile_pool(name="gpool", bufs=3, side="left")
        gps = tc.alloc_tile_pool(name="gps", bufs=2, space="PSUM")
        x2ps = tc.alloc_tile_pool(name="x2ps", bufs=2, space="PSUM")

        QW2 = 256  # fc2 token-group width: one accumulation group per bank
        for q in range(4):
            x2 = [x2ps.tile([P, 512], F32, tag="x2a", name=f"x2_{q}_{j}")
                  for j in range(2)]
            x2b = [x2ps.tile([P, 256], F32, tag="x2b", name=f"x2b_{q}_{j}")
                   for j in range(2)]
            for ff in range(FFT):
                w2 = wfc2s.tile([P, C], BF16, tag="w2", name=f"w2_{q}_{ff}")
                nc.sync.dma_start(w2[:], wfc2_d[ff * P:(ff + 1) * P, :])
                pg = gps.tile([P, QW2], F32, tag="G")
                for kt in range(KT):
                    nc.tensor.matmul(
                        pg[:],
                        wfc1[:, kt, ff * P:(ff + 1) * P],
                        h2_fm[:, kt, q * QW2:(q + 1) * QW2],
                        start=(kt == 0), stop=(kt == KT - 1),
                    )
                g_t = gpool.tile([P, QW2], BF16, tag="g")
                nc.scalar.activation(
                    out=g_t[:], in_=pg[:],
                    func=AF.Gelu, bias=bf1c[:, ff:ff + 1], scale=1.0,
                )
                for j in range(2):
                    nc.tensor.matmul(
                        x2[j][:],
                        g_t[:, j * P:(j + 1) * P],
                        w2[:, 0:512],
                        start=(ff == 0), stop=(ff == FFT - 1),
                    )
                    nc.tensor.matmul(
                        x2b[j][:],
                        g_t[:, j * P:(j + 1) * P],
                        w2[:, 512:768],
                        start=(ff == 0), stop=(ff == FFT - 1),
                    )
            for j in range(2):
                nt = 2 * q + j
                o_t = fstage.tile([P, C], F32, tag="fs", name=f"y_{nt}")
                nc.vector.tensor_add(
                    o_t[:, 0:512], x2[j][:], x1_tok[:, nt, 0:512]
                )
                nc.vector.tensor_add(
                    o_t[:, 512:768], x2b[j][:], x1_tok[:, nt, 512:768]
                )
                nc.vector.tensor_add(o_t[:], o_t[:], bf2b[:])
                nc.sync.dma_start(y_d[nt * P:(nt + 1) * P, :], o_t[:])
        x2ps.release()
        gps.release()
        x1p.release()
        gpool.release()
        wfc2s.release()
        wfc1p.release()
        h2p.release()
        fstage.release()
        lnp.release()
        consts.release()

    nc.compile()
    return nc


def _prep_inputs(inputs):
    """Host-side prep (exact refactoring of LN gains/biases into weights)."""
    f = lambda k: np.asarray(inputs[k], dtype=np.float32)
    x = f("x")
    w_qkv, w_proj, w_fc1, w_fc2 = f("w_qkv"), f("w_proj"), f("w_fc1"), f("w_fc2")
    ln1_g, ln1_b, ln2_g, ln2_b = f("ln1_g"), f("ln1_b"), f("ln2_g"), f("ln2_b")
    b_proj, b_fc1, b_fc2 = f("b_proj"), f("b_fc1"), f("b_fc2")

    bf = ml_dtypes.bfloat16
    w_qkv_e = ln1_g[:, None] * w_qkv
    qkv_bias = ln1_b @ w_qkv  # [2304]
    qk_bias = np.zeros((P, 2 * H), dtype=np.float32)
    for which in range(2):
        for h in range(H):
            qk_bias[0:DH, which * H + h] = qkv_bias[
                which * C + h * DH: which * C + (h + 1) * DH
            ]
    vb = qkv_bias[2 * C: 3 * C]  # v bias passes through softmax additively
    b_proj_e = b_proj + vb @ w_proj
    # head-aligned w_proj rows: block h rows 1..97 (row 0 pairs with colsum row)
    w_proj_p = np.zeros((H * P, C), dtype=np.float32)
    for h in range(H):
        w_proj_p[h * P + 1: h * P + 1 + DH, :] = w_proj[h * DH:(h + 1) * DH, :]
    w_fc1_e = ln2_g[:, None] * w_fc1
    b_fc1_e = b_fc1 + ln2_b @ w_fc1

    common = {
        "w_qkv_e": np.ascontiguousarray(w_qkv_e.astype(bf)),
        "w_proj_p": np.ascontiguousarray(w_proj_p.astype(bf)),
        "w_fc1_e": np.ascontiguousarray(w_fc1_e.astype(bf)),
        "w_fc2": np.ascontiguousarray(w_fc2.astype(bf)),
        "qk_bias": qk_bias,
        "b_proj_e": b_proj_e,
        "b_fc1_e": b_fc1_e,
        "b_fc2": b_fc2,
    }
    return [dict(common, x=np.ascontiguousarray(x[i])) for i in range(8)]


def kernel(**inputs):
    if "nc" not in _CACHED:
        _CACHED["nc"] = build()
    nc = _CACHED["nc"]
    in_maps = _prep_inputs(inputs)
    res = run_bass_kernel_spmd(nc, in_maps, core_ids=list(range(8)))
    out = np.stack([res.results[i]["y"] for i in range(8)], axis=0)
    return out.astype(np.float32)


# revision 16
# speedup vs baseline: 1.1603x; 1.1603x over previous
"""Transformer block (pre-norm attn + MLP) on 8 NeuronCores, data-parallel over batch.

Full inputs in, full outputs out. Each core processes one batch element
x[i] : [1024, 768] through an identical Bass/Tile kernel.

v2: single continuous PE stream. Differences vs v1:
  - warm-up matmul burst so HAM un-throttles before real work
  - LN1 pipelined per token tile with v-generation (PE dense from ~2us)
  - qk-gen interleaved into the attention pair loop (fills exp-wait bubbles)
  - attention pairs ordered nh-major so proj+LN2 of the first token half
    overlap the second half's ACT-bound stretch
  - softmax normalization via reciprocal_approx_fast (5x faster than
    reciprocal, which was 3.3us per pair and paced the whole phase)
  - LN rstd via Ln+Exp on ACT (stays in the natural_log_exp table set; no
    table thrash against the attention exp) and the normalize itself on DVE
  - MLP with 512-wide token groups (half the instruction count, bigger
    gelu batches); fc2 accumulates into a single [P, 6, 512] psum tile
  - x and x1 round-trip through DRAM instead of living in SBUF

Host-side exact refactoring (unchanged from v1):
  - LN gains fold into the following matmul weights: diag(g) @ W.
  - LN biases fold into: per-column bias on q/k, b_proj_eff, b_fc1_eff.
  - w_proj rows re-laid-out head-aligned (row 0 of each 128-block pairs with
    the attention colsum row; zero).
  - Weights cast to bf16 on host; matmuls bf16 x bf16 with fp32 PSUM.
"""
import numpy as np
import ml_dtypes

import concourse.bass as bass
from concourse import bacc, mybir
from concourse.bass_utils import run_bass_kernel_spmd
from concourse.masks import make_identity
from concourse.tile import TileContext

P = 128
N = 1024          # tokens per core (batch element)
C = 768           # model dim
H = 8             # heads
DH = C // H       # 96
DFF = 4 * C       # 3072
NT = N // P       # 8 token tiles
KT = C // P       # 6 feature tiles
FFT = DFF // P    # 24 ff tiles
NH = 2            # halves of the token axis for attention
NC_ = N // NH     # 512
EPS = 1e-5
SCALE = DH ** -0.5
VW = DH           # per-head v width (plus a leading ones column)
QW = 512          # MLP token-group width

F32 = mybir.dt.float32
BF16 = mybir.dt.bfloat16
AF = mybir.ActivationFunctionType
OP = mybir.AluOpType

_CACHED = {}


def build(taps=()):
    nc = bacc.Bacc("TRN2", debug=False)

    x_d = nc.dram_tensor("x", [N, C], F32, kind="ExternalInput")
    wqkv_d = nc.dram_tensor("w_qkv_e", [C, 3 * C], BF16, kind="ExternalInput")
    wproj_d = nc.dram_tensor("w_proj_p", [H * P, C], BF16, kind="ExternalInput")
    wfc1_d = nc.dram_tensor("w_fc1_e", [C, DFF], BF16, kind="ExternalInput")
    wfc2_d = nc.dram_tensor("w_fc2", [DFF, C], BF16, kind="ExternalInput")
    qkb_d = nc.dram_tensor("qk_bias", [P, 2 * H], F32, kind="ExternalInput")
    bp_d = nc.dram_tensor("b_proj_e", [C], F32, kind="ExternalInput")
    bf1_d = nc.dram_tensor("b_fc1_e", [DFF], F32, kind="ExternalInput")
    bf2_d = nc.dram_tensor("b_fc2", [C], F32, kind="ExternalInput")
    y_d = nc.dram_tensor("y", [N, C], F32, kind="ExternalOutput")
    dbg_cs = nc.dram_tensor("dbg_cs", [2 * H, NC_], F32, kind="ExternalOutput")
    dbg_csr = nc.dram_tensor("dbg_csr", [2 * H, NC_], F32, kind="ExternalOutput")

    tap_d = {}
    for name, shape, dt in [
        ("h_fm", [C, N], BF16),
        ("q_fm", [H * P, N], BF16),
        ("k_fm", [H * P, N], BF16),
        ("v_ext", [N, H * (VW + 1)], BF16),
        ("o_fm", [H * P, N], BF16),
        ("x1", [N, C], F32),
        ("h2_fm", [C, N], BF16),
    ]:
        if name in taps:
            tap_d[name] = nc.dram_tensor(
                "tap_" + name, shape, dt, kind="ExternalOutput"
            )

    def bcast_row(dram_t, width):
        return bass.AP(tensor=dram_t, offset=0, ap=[[0, P], [1, width]])

    with TileContext(nc) as tc:
        # ---------------- pools: LEFT stack ----------------
        consts = tc.alloc_tile_pool(name="consts", bufs=1, side="left")
        lnp = tc.alloc_tile_pool(name="lnp", bufs=2, side="left")
        fstage = tc.alloc_tile_pool(name="fstage", bufs=3, side="left")
        h2p = tc.alloc_tile_pool(name="h2p", bufs=1, side="left")
        wqkvp = tc.alloc_tile_pool(name="wqkvp", bufs=1, side="left")
        hfmp = tc.alloc_tile_pool(name="hfmp", bufs=1, side="left")
        wqkvvp = tc.alloc_tile_pool(name="wqkvvp", bufs=1, side="left")
        # ---------------- pools: RIGHT stack ----------------
        x1p = tc.alloc_tile_pool(name="x1p", bufs=1, side="right")
        qkvpool = tc.alloc_tile_pool(name="qkvpool", bufs=1, side="right")
        opool = tc.alloc_tile_pool(name="opool", bufs=1, side="right")
        epool = tc.alloc_tile_pool(name="epool", bufs=2, side="right")
        rrow = tc.alloc_tile_pool(name="rrow", bufs=2, side="right")
        # ---------------- pools: PSUM ----------------
        ps = tc.alloc_tile_pool(name="ps", bufs=3, space="PSUM")
        sps = tc.alloc_tile_pool(name="sps", bufs=2, space="PSUM")

        # ---------- constants ----------
        ident = consts.tile([P, P], BF16)
        make_identity(nc, ident)
        eps_t = consts.tile([P, 1], F32)
        nc.vector.memset(eps_t, EPS)
        qkb = consts.tile([P, 2 * H], F32)
        nc.gpsimd.dma_start(qkb[:], qkb_d[:, :])
        bf1c = consts.tile([P, FFT], F32)
        nc.gpsimd.dma_start(bf1c[:], bf1_d.rearrange("(t p) -> p t", p=P))
        bpb = consts.tile([P, C], F32)
        nc.gpsimd.dma_start(bpb[:], bcast_row(bp_d, C))
        bf2b = consts.tile([P, C], F32)
        nc.gpsimd.dma_start(bf2b[:], bcast_row(bf2_d, C))

        # ---------- big tiles ----------
        h_fm = hfmp.tile([P, KT, N], BF16)
        wqkv = wqkvp.tile([P, KT, 2 * C], BF16)
        wqkv_v = wqkvvp.tile([P, KT, C], BF16)
        h2_fm = h2p.tile([P, KT, N], BF16)
        x1_tok = x1p.tile([P, NT, C], F32)
        q_fm = qkvpool.tile([P, H, N], BF16)
        k_fm = qkvpool.tile([P, H, N], BF16)
        v_ext = qkvpool.tile([P, NT, H, VW + 1], BF16)
        o_fm = opool.tile([P, H, N], BF16)

        # ---------- weight DMAs (sync queue, in need-order) ----------
        # v columns of wqkv first (phase A consumes them), then q, then k.
        wqkv_r = wqkv_d.rearrange("(kt p) o -> p kt o", p=P)
        for c0 in range(0, C, 512):
            cw = min(512, C - c0)
            nc.sync.dma_start(
                wqkv_v[:, :, c0:c0 + cw], wqkv_r[:, :, 2 * C + c0:2 * C + c0 + cw]
            )
        for c0 in range(0, 2 * C, 512):
            nc.sync.dma_start(
                wqkv[:, :, c0:c0 + 512], wqkv_r[:, :, c0:c0 + 512]
            )

        # ---------- pad memsets (DVE/gpsimd, once) ----------
        nc.vector.memset(q_fm[DH:P, :, :], 0.0)
        nc.vector.memset(k_fm[DH:P, :, :], 0.0)
        nc.vector.memset(o_fm[DH:P, :, :], 0.0)
        nc.gpsimd.memset(v_ext[:, :, :, 0], 1.0)

        # ---------- HAM warm-up burst (~3.5us of tiny matmuls) ----------
        warm = ps.tile([P, P], F32, tag="ps")
        for i in range(36):
            nc.tensor.matmul(warm[:, :P], ident[:], ident[:],
                             start=True, stop=True)

        # ---------- helpers ----------
        def ln_stats(x_ap, tag):
            st = lnp.tile([P, 3, nc.vector.BN_STATS_DIM], F32, tag="bnst")
            for i in range(3):
                nc.vector.bn_stats(
                    out=st[:, i, :], in_=x_ap[:, i * 256:(i + 1) * 256]
                )
            mv = lnp.tile([P, nc.vector.BN_AGGR_DIM], F32, tag=tag)
            nc.vector.bn_aggr(out=mv[:], in_=st[:])
            return mv

        def ln_norm(x_ap, mv, h_tile, tag):
            # rstd = 1/sqrt(var + eps): Sqrt on ACT (single table set),
            # reciprocal on DVE ([P,1] is cheap there)
            rstd = lnp.tile([P, 1], F32, tag=tag)
            nc.scalar.activation(
                out=rstd[:], in_=mv[:, 1:2], func=AF.Sqrt,
                bias=eps_t[:], scale=1.0,
            )
            nc.vector.reciprocal(out=rstd[:], in_=rstd[:])
            nc.vector.tensor_scalar(
                out=h_tile[:], in0=x_ap[:],
                scalar1=mv[:, 0:1], scalar2=rstd[:],
                op0=OP.subtract, op1=OP.mult,
            )

        def layernorm_tile(x_ap, h_tile):
            mv = ln_stats(x_ap, "bnmv")
            ln_norm(x_ap, mv, h_tile, "rstd")

        def transpose_into(h_tile, dst_fm, nt):
            for kt in range(KT):
                tp = ps.tile([P, P], BF16, tag="ps", name=f"tp_{nt}_{kt}")
                nc.tensor.transpose(
                    tp[:], h_tile[:, kt * P:(kt + 1) * P], ident[:]
                )
                nc.vector.tensor_copy(
                    dst_fm[:, kt, nt * P:(nt + 1) * P], tp[:]
                )

        # ---------- phase A: LN1 + transpose + v-gen, per token tile ----------
        for nt in range(NT):
            x_t = fstage.tile([P, C], F32, tag="fs", name=f"xa_{nt}")
            nc.sync.dma_start(x_t[:], x_d[nt * P:(nt + 1) * P, :])
            h_t = lnp.tile([P, C], BF16, tag="h1")
            layernorm_tile(x_t[:], h_t)
            transpose_into(h_t, h_fm, nt)
            for half in range(2):  # 4 heads (384 cols) per psum
                pv = ps.tile([P, NC_], F32, tag="ps", name=f"pv_{nt}_{half}")
                c0 = half * 4 * DH
                for kt in range(KT):
                    nc.tensor.matmul(
                        pv[:, :4 * DH],
                        h_fm[:, kt, nt * P:(nt + 1) * P],
                        wqkv_v[:, kt, c0:c0 + 4 * DH],
                        start=(kt == 0), stop=(kt == KT - 1),
                    )
                nc.vector.tensor_copy(
                    v_ext[:, nt, half * 4:(half + 1) * 4, 1:VW + 1],
                    pv[:, :4 * DH].rearrange("p (h d) -> p h d", d=DH),
                )

        if "h_fm" in tap_d:
            nc.sync.dma_start(
                tap_d["h_fm"].rearrange("(kt p) n -> p kt n", p=P), h_fm[:]
            )
        if "v_ext" in tap_d:
            nc.sync.dma_start(
                tap_d["v_ext"].rearrange(
                    "(nt p) (h w) -> p nt h w", p=P, w=VW + 1),
                v_ext[:],
            )

        wqkvvp.release()

        # ---------- phase B/C: qk-gen interleaved with attention ----------
        def emit_qk_group(h, which, nh):
            """One psum group: q or k for head h, token half nh."""
            col0 = which * C + h * DH
            dst = q_fm if which == 0 else k_fm
            pq = ps.tile([P, NC_], F32, tag="ps", name=f"qk_{h}_{which}_{nh}")
            for kt in range(KT):
                nc.tensor.matmul(
                    pq[:DH, :],
                    wqkv[:, kt, col0:col0 + DH],
                    h_fm[:, kt, nh * NC_:(nh + 1) * NC_],
                    start=(kt == 0), stop=(kt == KT - 1),
                )
            nc.vector.tensor_scalar_add(
                dst[:DH, h, nh * NC_:(nh + 1) * NC_],
                pq[:DH, :],
                qkb[:DH, which * H + h:which * H + h + 1],
            )

        def emit_scores_part(h, nh, mt2, e_t):
            ps_s = sps.tile([P, 2, NC_], F32, tag="S",
                            name=f"s_{h}_{nh}_{mt2}")
            for sub in range(2):
                nc.tensor.matmul(
                    ps_s[:, sub, :],
                    k_fm[:, h, (2 * mt2 + sub) * P:(2 * mt2 + sub + 1) * P],
                    q_fm[:, h, nh * NC_:(nh + 1) * NC_],
                    start=True, stop=True,
                )
            nc.scalar.activation(
                out=e_t[:, 2 * mt2:2 * mt2 + 2, :], in_=ps_s[:],
                func=AF.Exp, bias=0.0, scale=SCALE,
            )

        def emit_pv(h, nh, e_t):
            po = ps.tile([P, NC_], F32, tag="ps", name=f"po_{h}_{nh}")
            for mt in range(NT):
                nc.tensor.matmul(
                    po[:VW + 1, :],
                    v_ext[:, mt, h, :],
                    e_t[:, mt, :],
                    start=(mt == 0), stop=(mt == NT - 1),
                )
            # softmax normalize: rinv = 1/colsum, broadcast, multiply
            cs = rrow.tile([1, NC_], F32, tag="cs", name=f"cs_{h}_{nh}")
            nc.vector.tensor_copy(cs[0:1, :], po[0:1, :])
            nc.sync.dma_start(dbg_csr[nh * H + h:nh * H + h + 1, :], cs[0:1, :])
            nc.vector.reciprocal_approx_fast(out=cs[0:1, :], in_=cs[0:1, :])
            nc.sync.dma_start(dbg_cs[nh * H + h:nh * H + h + 1, :], cs[0:1, :])
            rb = rrow.tile([P, NC_], F32, tag="rb", name=f"rb_{h}_{nh}")
            nc.gpsimd.partition_broadcast(rb[:VW + 1, :], cs[0:1, :])
            nc.vector.tensor_mul(
                o_fm[0:VW + 1, h, nh * NC_:(nh + 1) * NC_],
                po[0:VW + 1, :], rb[0:VW + 1, :],
            )

        def emit_proj(nt):
            x_t2 = fstage.tile([P, C], F32, tag="fs", name=f"xb_{nt}")
            nc.sync.dma_start(x_t2[:], x_d[nt * P:(nt + 1) * P, :])
            x1_t = x1_tok[:, nt, :]
            for c0, cw in ((0, 512), (512, 256)):
                pj = ps.tile([P, NC_], F32, tag="ps", name=f"pj_{nt}_{c0}")
                for hb in range(H):
                    nc.tensor.matmul(
                        pj[:, :cw],
                        o_fm[:, hb, nt * P:(nt + 1) * P],
                        wproj[:, hb, c0:c0 + cw],
                        start=(hb == 0), stop=(hb == H - 1),
                    )
                nc.vector.tensor_add(
                    x1_t[:, c0:c0 + cw], pj[:, :cw], x_t2[:, c0:c0 + cw]
                )
            nc.vector.tensor_add(x1_t[:], x1_t[:], bpb[:])
            if "x1" in tap_d:
                nc.sync.dma_start(
                    tap_d["x1"][nt * P:(nt + 1) * P, :], x1_t[:]
                )
            return x1_t

        ln2_mv = {}

        def emit_ln2_stats(nt, x1_t):
            ln2_mv[nt] = ln_stats(x1_t[:], f"mv2_{nt}")

        def emit_ln2_norm(nt):
            h_t = lnp.tile([P, C], BF16, tag="h1", name=f"h2_{nt}")
            ln_norm(x1_tok[:, nt, :], ln2_mv[nt], h_t, f"rstd2_{nt}")
            transpose_into(h_t, h2_fm, nt)

        pairs = [(h, nh) for nh in range(NH) for h in range(H)]
        # scores(h, nh) needs q(h, nh) and k(h, BOTH halves): emit per head
        # [q-nh0, k-nh0, k-nh1, q-nh1], 4 groups per pair iteration.
        fill_q = [g for h in range(2, H)
                  for g in [(h, 0, 0), (h, 1, 0), (h, 1, 1), (h, 0, 1)]]
        qk_done = set()

        def emit_qk_group_tracked(h, wh, nh):
            emit_qk_group(h, wh, nh)
            qk_done.add((h, wh, nh))

        for h in range(2):
            for nh in range(NH):
                for wh in range(2):
                    emit_qk_group_tracked(h, wh, nh)

        prev = None
        wprojp = None
        wproj = None
        for i, (h, nh) in enumerate(pairs):
            # scores for (h, nh) need q(h, nh) and k(h, 0) AND k(h, 1)
            assert {(h, 0, nh), (h, 1, 0), (h, 1, 1)} <= qk_done, (h, nh)
            e_t = epool.tile([P, NT, NC_], BF16, tag="E", name=f"e_{h}_{nh}")
            for mt2 in range(NT // 2):
                emit_scores_part(h, nh, mt2, e_t)
                if fill_q:
                    hq, wq, nq = fill_q.pop(0)
                    emit_qk_group_tracked(hq, wq, nq)
            if not fill_q and wprojp is None:
                # qk-gen fully emitted: free h_fm + wqkv; bring in w_proj, wfc1
                hfmp.release()
                wqkvp.release()
                wprojp = tc.alloc_tile_pool(name="wprojp", bufs=1, side="right")
                wproj = wprojp.tile([P, H, C], BF16)
                nc.sync.dma_start(
                    wproj[:], wproj_d.rearrange("(hb p) c -> p hb c", p=P)
                )
                wfc1p = tc.alloc_tile_pool(name="wfc1p", bufs=1, side="left")
                wfc1 = wfc1p.tile([P, KT, DFF], BF16)
                nc.sync.dma_start(
                    wfc1[:], wfc1_d.rearrange("(kt p) f -> p kt f", p=P)
                )
            if prev is not None:
                emit_pv(*prev)
            prev = (h, nh, e_t)
            # after all nh=0 pairs are normalized, overlap proj+LN2 of the
            # first token half with the second half's attention
            if 9 <= i <= 12:
                nt = i - 9
                emit_ln2_stats(nt, emit_proj(nt))
        emit_pv(*prev)
        for nt in range(4, NT):
            emit_ln2_stats(nt, emit_proj(nt))
        # all exp activity is done: batched LN2 sqrt (one table load),
        # normalizes + transposes overlap the proj tail on PE
        for nt in range(NT):
            emit_ln2_norm(nt)

        if "q_fm" in tap_d:
            nc.sync.dma_start(
                tap_d["q_fm"].rearrange("(h p) n -> p h n", p=P), q_fm[:]
            )
        if "k_fm" in tap_d:
            nc.sync.dma_start(
                tap_d["k_fm"].rearrange("(h p) n -> p h n", p=P), k_fm[:]
            )
        if "o_fm" in tap_d:
            nc.sync.dma_start(
                tap_d["o_fm"].rearrange("(h p) n -> p h n", p=P), o_fm[:]
            )
        if "h2_fm" in tap_d:
            nc.sync.dma_start(
                tap_d["h2_fm"].rearrange("(kt p) n -> p kt n", p=P), h2_fm[:]
            )

        # ---------- release attention-era pools ----------
        sps.release()
        ps.release()
        wprojp.release()
        rrow.release()
        epool.release()
        opool.release()
        qkvpool.release()

        # ---------- phase D: MLP ----------
        wfc2s = tc.alloc_tile_pool(name="wfc2s", bufs=8, side="left")
        gpool = tc.alloc_tile_pool(name="gpool", bufs=3, side="left")
        gps = tc.alloc_tile_pool(name="gps", bufs=2, space="PSUM")
        x2ps = tc.alloc_tile_pool(name="x2ps", bufs=2, space="PSUM")

        QW2 = 256  # fc2 token-group width: one accumulation group per bank
        for q in range(4):
            x2 = [x2ps.tile([P, 512], F32, tag="x2a", name=f"x2_{q}_{j}")
                  for j in range(2)]
            x2b = [x2ps.tile([P, 256], F32, tag="x2b", name=f"x2b_{q}_{j}")
                   for j in range(2)]
            for ff in range(FFT):
                w2 = wfc2s.tile([P, C], BF16, tag="w2", name=f"w2_{q}_{ff}")
                nc.sync.dma_start(w2[:], wfc2_d[ff * P:(ff + 1) * P, :])
                pg = gps.tile([P, QW2], F32, tag="G")
                for kt in range(KT):
                    nc.tensor.matmul(
                        pg[:],
                        wfc1[:, kt, ff * P:(ff + 1) * P],
                        h2_fm[:, kt, q * QW2:(q + 1) * QW2],
                        start=(kt == 0), stop=(kt == KT - 1),
                    )
                g_t = gpool.tile([P, QW2], BF16, tag="g")
                nc.scalar.activation(
                    out=g_t[:], in_=pg[:],
                    func=AF.Gelu, bias=bf1c[:, ff:ff + 1], scale=1.0,
                )
                for j in range(2):
                    nc.tensor.matmul(
                        x2[j][:],
                        g_t[:, j * P:(j + 1) * P],
                        w2[:, 0:512],
                        start=(ff == 0), stop=(ff == FFT - 1),
                    )
                    nc.tensor.matmul(
                        x2b[j][:],
                        g_t[:, j * P:(j + 1) * P],
                        w2[:, 512:768],
                        start=(ff == 0), stop=(ff == FFT - 1),
                    )
            for j in range(2):
                nt = 2 * q + j
                o_t = fstage.tile([P, C], F32, tag="fs", name=f"y_{nt}")
                nc.vector.tensor_add(
                    o_t[:, 0:512], x2[j][:], x1_tok[:, nt, 0:512]
                )
                nc.vector.tensor_add(
                    o_t[:, 512:768], x2b[j][:], x1_tok[:, nt, 512:768]
                )
                nc.vector.tensor_add(o_t[:], o_t[:], bf2b[:])
                nc.sync.dma_start(y_d[nt * P:(nt + 1) * P, :], o_t[:])
        x2ps.release()
        gps.release()
        x1p.release()
        gpool.release()
        wfc2s.release()
        wfc1p.release()
        h2p.release()
        fstage.release()
        lnp.release()
        consts.release()

    nc.compile()
    return nc


def _prep_inputs(inputs):
    """Host-side prep (exact refactoring of LN gains/biases into weights)."""
    f = lambda k: np.asarray(inputs[k], dtype=np.float32)
    x = f("x")
    w_qkv, w_proj, w_fc1, w_fc2 = f("w_qkv"), f("w_proj"), f("w_fc1"), f("w_fc2")
    ln1_g, ln1_b, ln2_g, ln2_b = f("ln1_g"), f("ln1_b"), f("ln2_g"), f("ln2_b")
    b_proj, b_fc1, b_fc2 = f("b_proj"), f("b_fc1"), f("b_fc2")

    bf = ml_dtypes.bfloat16
    w_qkv_e = ln1_g[:, None] * w_qkv
    qkv_bias = ln1_b @ w_qkv  # [2304]
    qk_bias = np.zeros((P, 2 * H), dtype=np.float32)
    for which in range(2):
        for h in range(H):
            qk_bias[0:DH, which * H + h] = qkv_bias[
                which * C + h * DH: which * C + (h + 1) * DH
            ]
    vb = qkv_bias[2 * C: 3 * C]  # v bias passes through softmax additively
    b_proj_e = b_proj + vb @ w_proj
    # head-aligned w_proj rows: block h rows 1..97 (row 0 pairs with colsum row)
    w_proj_p = np.zeros((H * P, C), dtype=np.float32)
    for h in range(H):
        w_proj_p[h * P + 1: h * P + 1 + DH, :] = w_proj[h * DH:(h + 1) * DH, :]
    w_fc1_e = ln2_g[:, None] * w_fc1
    b_fc1_e = b_fc1 + ln2_b @ w_fc1

    common = {
        "w_qkv_e": np.ascontiguousarray(w_qkv_e.astype(bf)),
        "w_proj_p": np.ascontiguousarray(w_proj_p.astype(bf)),
        "w_fc1_e": np.ascontiguousarray(w_fc1_e.astype(bf)),
        "w_fc2": np.ascontiguousarray(w_fc2.astype(bf)),
        "qk_bias": qk_bias,
        "b_proj_e": b_proj_e,
        "b_fc1_e": b_fc1_e,
        "b_fc2": b_fc2,
    }
    return [dict(common, x=np.ascontiguousarray(x[i])) for i in range(8)]


def kernel(**inputs):
    if "nc" not in _CACHED:
        _CACHED["nc"] = build()
    nc = _CACHED["nc"]
    in_maps = _prep_inputs(inputs)
    res = run_bass_kernel_spmd(nc, in_maps, core_ids=list(range(8)))
    out = np.stack([res.results[i]["y"] for i in range(8)], axis=0)
    return out.astype(np.float32)
